# revision 1
# baseline (speedup 1.0000x reference)
"""Trainium2 Bass kernel for nn_LittleBitParallelLinear.

Computes y = ((x * h_in) @ sign(V)) * s @ sign(U).T * h_out with
sign(z) = +1 for z >= 0, -1 otherwise.

Strategy: token-parallel across 8 NeuronCores. Core i handles tokens
[i*1024, (i+1)*1024); weights are replicated. Inside each core everything
is computed transposed (tokens on the matmul free dim) so that h_in, s and
h_out all become per-partition scales:

    aT  = (xT * h_in)            [IN, TOK]   bf16, SBUF-resident
    tT  = (sign(V).T @ aT) * s   [RANK, TOK] bf16, SBUF-resident
    yT  = (sign(U) @ tT) * h_out [OUT, TOK]  fp32, streamed to DRAM

Matmuls run in bf16 (sign weights are exactly representable; activations
round to ~0.4% rel err). The host pre-transposes x and u and casts the
big tensors to bf16 so DMA traffic is halved; the sign() itself is
computed on-device.
"""

import numpy as np
import ml_dtypes

P = 128
TOKENS, IN, OUT, RANK = 8192, 4096, 4096, 2048
N_CORES = 8
TOK = TOKENS // N_CORES  # tokens per core
KI = IN // P             # 32 contraction subtiles for mm1
KR = RANK // P           # 16 contraction subtiles for mm2
MR = RANK // P           # 16 rank row-blocks (mm1 output)
MO = OUT // P            # 32 out row-blocks (mm2 output)
FREE = 512               # PSUM bank free-dim (fp32)
NT = TOK // FREE         # 2 free chunks of the token dim

_cache = {}


def _build(reps=1):
    import concourse.bacc as bacc
    import concourse.mybir as mybir
    import concourse.tile as tile

    f32 = mybir.dt.float32
    bf16 = mybir.dt.bfloat16
    Sign = mybir.ActivationFunctionType.Sign
    Copy = mybir.ActivationFunctionType.Copy

    nc = bacc.Bacc("TRN2", target_bir_lowering=False, debug=False)

    xT = nc.dram_tensor("xT", [IN, TOK], bf16, kind="ExternalInput").ap()
    # weights arrive pre-tiled: block m is contiguous [P, K_sub, P]
    v_ = nc.dram_tensor("v", [MR, P, KI, P], bf16, kind="ExternalInput").ap()
    uT = nc.dram_tensor("uT", [MO, P, KR, P], bf16, kind="ExternalInput").ap()
    s_ = nc.dram_tensor("s", [P, KR], f32, kind="ExternalInput").ap()
    hi = nc.dram_tensor("h_in", [P, KI], f32, kind="ExternalInput").ap()
    ho = nc.dram_tensor("h_out", [P, MO], f32, kind="ExternalInput").ap()
    yT = nc.dram_tensor("yT", [OUT, TOK], f32, kind="ExternalOutput").ap()

    with tile.TileContext(nc) as tc:
      for rep in range(reps):
        with (
            tc.tile_pool(name=f"const{rep}", bufs=1) as const,
            tc.tile_pool(name=f"aT{rep}", bufs=1) as apool,
            tc.tile_pool(name=f"tT{rep}", bufs=1) as tpool,
            tc.tile_pool(name=f"xin{rep}", bufs=3) as xpool,
            tc.tile_pool(name=f"vin{rep}", bufs=3) as vpool,
            tc.tile_pool(name=f"bv{rep}", bufs=4) as bvpool,
            tc.tile_pool(name=f"uin{rep}", bufs=2) as upool,
            tc.tile_pool(name=f"bu{rep}", bufs=2) as bupool,
            tc.tile_pool(name=f"yout{rep}", bufs=3) as ypool,
            tc.tile_pool(name=f"psum{rep}", bufs=8, space="PSUM") as psum,
        ):

            # aT = xT * h_in, bf16, fully SBUF-resident [P, KI, TOK]
            # Interleave the x-tile loads with the v-weight loads in issue
            # order so the first weight blocks aren't queued behind all of x.
            aT = apool.tile([P, KI, TOK], bf16)
            x3 = xT.rearrange("(ko p) t -> p ko t", p=P)

            bv_tiles = {}

            def load_bv(m, nchunk=1):
                vt = vpool.tile([P, KI, P], bf16, name=f"vt{rep}_{m}", tag="vt")
                step = KI // nchunk
                for c in range(0, KI, step):
                    nc.sync.dma_start(vt[:, c : c + step], v_[m, :, c : c + step])
                bv = bvpool.tile([P, KI, P], bf16, name=f"bv{rep}_{m}", tag="bv")
                for c in range(0, KI, 8):
                    nc.scalar.activation(bv[:, c : c + 8], vt[:, c : c + 8], Sign)
                bv_tiles[m] = bv

            load_bv(0, nchunk=4)
            # consts: pre-tiled on host, contiguous small DMAs
            hi_sb = const.tile([P, KI], f32)
            nc.sync.dma_start(hi_sb, hi)
            load_bv(1)
            s_sb = const.tile([P, KR], f32)
            nc.sync.dma_start(s_sb, s_)
            ho_sb = const.tile([P, MO], f32)
            nc.sync.dma_start(ho_sb, ho)
            for k in range(KI):
                xt = xpool.tile([P, TOK], bf16, name=f"xt{rep}_{k}", tag="xt")
                nc.sync.dma_start(xt, x3[:, k])
                nc.vector.tensor_scalar_mul(aT[:, k], xt, hi_sb[:, k : k + 1])

            # tT = (sign(V).T @ aT) * s, bf16, SBUF-resident [P, KR, TOK]
            tT = tpool.tile([P, KR, TOK], bf16)
            for m in range(MR):
                if 2 + m <= MR - 1:
                    load_bv(2 + m)
                bv = bv_tiles.pop(m)
                pss = [
                    psum.tile([P, FREE], f32, name=f"ps1_{rep}_{m}_{n}", tag="ps")
                    for n in range(NT)
                ]
                for k in range(KI):
                    for n in range(NT):
                        nc.tensor.matmul(
                            pss[n],
                            lhsT=bv[:, k],
                            rhs=aT[:, k, n * FREE : (n + 1) * FREE],
                            start=(k == 0),
                            stop=(k == KI - 1),
                        )
                for n in range(NT):
                    nc.scalar.activation(
                        tT[:, m, n * FREE : (n + 1) * FREE],
                        pss[n],
                        Copy,
                        scale=s_sb[:, m : m + 1],
                    )

            # yT = (sign(U) @ tT) * h_out, fp32, streamed out
            y3 = yT.rearrange("(mo p) t -> p mo t", p=P)
            for m in range(MO):
                ut = upool.tile([P, KR, P], bf16)
                nc.sync.dma_start(ut, uT[m])
                bu = bupool.tile([P, KR, P], bf16)
                for c in range(0, KR, 8):
                    nc.scalar.activation(bu[:, c : c + 8], ut[:, c : c + 8], Sign)
                pss = [
                    psum.tile([P, FREE], f32, name=f"ps2_{rep}_{m}_{n}", tag="ps")
                    for n in range(NT)
                ]
                for k in range(KR):
                    for n in range(NT):
                        nc.tensor.matmul(
                            pss[n],
                            lhsT=bu[:, k],
                            rhs=tT[:, k, n * FREE : (n + 1) * FREE],
                            start=(k == 0),
                            stop=(k == KR - 1),
                        )
                yst = ypool.tile([P, TOK], f32)
                for n in range(NT):
                    nc.scalar.activation(
                        yst[:, n * FREE : (n + 1) * FREE],
                        pss[n],
                        Copy,
                        scale=ho_sb[:, m : m + 1],
                    )
                nc.sync.dma_start(y3[:, m], yst)

    nc.compile()
    return nc


def _run(inputs, trace=False):
    from concourse.bass_utils import run_bass_kernel_spmd

    if "nc" not in _cache:
        _cache["nc"] = _build()
    nc = _cache["nc"]

    x = inputs["x"]
    u = inputs["u"]
    v = inputs["v"]
    def ptile(vec, o):
        return np.ascontiguousarray(
            np.asarray(vec, dtype=np.float32).reshape(o, P).T
        )

    s = ptile(inputs["s"], KR)
    h_in = ptile(inputs["h_in"], KI)
    h_out = ptile(inputs["h_out"], MO)

    bf = ml_dtypes.bfloat16
    # pre-tile weights so each 128-wide block is a contiguous DMA:
    # v_t[m, p, k, r] = v[k*128+p, m*128+r]; u_t[m, p, k, o] = u[m*128+o, k*128+p]
    v_bf = np.ascontiguousarray(
        np.asarray(v).reshape(KI, P, MR, P).transpose(2, 1, 0, 3)
    ).astype(bf)
    uT_bf = np.ascontiguousarray(
        np.asarray(u).T.reshape(KR, P, MO, P).transpose(2, 1, 0, 3)
    ).astype(bf)

    in_maps = []
    for i in range(N_CORES):
        xT_i = np.ascontiguousarray(x[i * TOK : (i + 1) * TOK, :].T).astype(bf)
        in_maps.append(
            {
                "xT": xT_i,
                "v": v_bf,
                "uT": uT_bf,
                "s": s,
                "h_in": h_in,
                "h_out": h_out,
            }
        )

    _cache["in_maps"] = in_maps
    res = run_bass_kernel_spmd(
        nc, in_maps, core_ids=list(range(N_CORES)), trace=trace
    )

    y = np.empty((TOKENS, OUT), dtype=np.float32)
    for i in range(N_CORES):
        y[i * TOK : (i + 1) * TOK, :] = res.results[i]["yT"].T
    return y, res


def kernel(**inputs):
    y, _ = _run(inputs, trace=False)
    return y



# revision 3
# speedup vs baseline: 1.0092x; 1.0092x over previous
"""Trainium2 Bass kernel for nn_LittleBitParallelLinear.

Computes y = ((x * h_in) @ sign(V)) * s @ sign(U).T * h_out with
sign(z) = +1 for z >= 0, -1 otherwise.

Strategy: fold the whole weight chain into a single dense matrix on the
host:  W = diag(h_in) @ sign(V) @ diag(s) @ sign(U).T @ diag(h_out).
Because RANK == IN/2 == OUT/2, the folded matmul x @ W has exactly the
same FLOP count as the two-matmul form (IN*OUT == IN*RANK + RANK*OUT),
but the device kernel becomes a single streaming GEMM with no Sign
activations, no intermediate stage, and half the dependency depth.

Token-parallel across 8 NeuronCores: core i handles tokens
[i*1024, (i+1)*1024); W is replicated. Inside each core the compute is
transposed (tokens on the matmul free dim):

    yT = W.T @ xT    [OUT, TOK]  fp16, streamed to DRAM

x and W are fp16 (W pre-scaled by 1/4 for range headroom; the host
multiplies the output by 4). PSUM accumulates fp32; the fp32->fp16
output cast rides the scalar engine. DMA instruction count is kept low
(x in 4 chunks, W/y in 2-block chunks) — per-DMA fixed costs on the SP
queue are significant on this hardware — and all DMAs stay on the SP
queue (the Activation-engine DGE queue measures ~2x slower).
"""

import numpy as np

P = 128
TOKENS, IN, OUT = 8192, 4096, 4096
N_CORES = 8
TOK = TOKENS // N_CORES   # 1024 tokens per core
KI = IN // P              # 32 contraction subtiles
MO = OUT // P             # 32 output row-blocks
FREE = 512                # PSUM bank free-dim (fp32)
NT = TOK // FREE          # 2 free chunks
XCHUNK = 8                # k-subtiles per x DMA
WCHUNK = 2                # output blocks per W DMA
YCHUNK = 2                # output blocks per y DMA
J = MO // WCHUNK

_cache = {}


def _build():
    import concourse.bacc as bacc
    import concourse.mybir as mybir
    import concourse.tile as tile

    f32 = mybir.dt.float32
    f16 = mybir.dt.float16
    Copy = mybir.ActivationFunctionType.Copy

    nc = bacc.Bacc("TRN2", target_bir_lowering=False, debug=False)

    xT = nc.dram_tensor("xT", [IN, TOK], f16, kind="ExternalInput").ap()
    # W pre-tiled on host: w[j, p, k, c, o] = W[k*128+p, (j*WCHUNK+c)*128+o]
    w_ = nc.dram_tensor(
        "w", [J, P, KI, WCHUNK, P], f16, kind="ExternalInput"
    ).ap()
    yT = nc.dram_tensor("yT", [OUT, TOK], f16, kind="ExternalOutput").ap()

    with tile.TileContext(nc) as tc:
        with (
            tc.tile_pool(name="x", bufs=1) as xpool,
            tc.tile_pool(name="w", bufs=3) as wpool,
            tc.tile_pool(name="y", bufs=4) as ypool,
            tc.tile_pool(name="ps", bufs=8, space="PSUM") as psum,
        ):
            xs = xpool.tile([P, KI, TOK], f16)
            x3 = xT.rearrange("(k p) t -> p k t", p=P)
            y3 = yT.rearrange("(m p) t -> p m t", p=P)

            w_tiles = {}

            def load_w(j):
                wt = wpool.tile(
                    [P, KI, WCHUNK, P], f16, name=f"wt{j}", tag="wt"
                )
                nc.sync.dma_start(wt, w_[j])
                w_tiles[j] = wt

            load_w(0)
            next_wj = 1
            for kc in range(0, KI, XCHUNK):
                nc.sync.dma_start(
                    xs[:, kc : kc + XCHUNK], x3[:, kc : kc + XCHUNK]
                )
                if kc == 0 and next_wj < J:
                    load_w(next_wj)
                    next_wj += 1

            yt = None
            for m in range(MO):
                j, c = divmod(m, WCHUNK)
                if c == 0 and next_wj <= min(j + 2, J - 1):
                    load_w(next_wj)
                    next_wj += 1
                wt = w_tiles[j]
                pss = [
                    psum.tile([P, FREE], f32, name=f"ps_{m}_{n}", tag="ps")
                    for n in range(NT)
                ]
                for k in range(KI):
                    for n in range(NT):
                        nc.tensor.matmul(
                            pss[n],
                            lhsT=wt[:, k, c],
                            rhs=xs[:, k, n * FREE : (n + 1) * FREE],
                            start=(k == 0),
                            stop=(k == KI - 1),
                        )
                yc = m % YCHUNK
                if yc == 0:
                    yt = ypool.tile(
                        [P, YCHUNK, TOK], f16, name=f"yt_{m}", tag="yt"
                    )
                for n in range(NT):
                    nc.scalar.activation(
                        yt[:, yc, n * FREE : (n + 1) * FREE], pss[n], Copy
                    )
                if yc == YCHUNK - 1:
                    nc.sync.dma_start(y3[:, m - YCHUNK + 1 : m + 1], yt)
                if c == WCHUNK - 1:
                    w_tiles.pop(j)

    nc.compile()
    return nc


def _run(inputs, trace=False):
    from concourse.bass_utils import run_bass_kernel_spmd

    if "nc" not in _cache:
        _cache["nc"] = _build()
    nc = _cache["nc"]

    x = np.asarray(inputs["x"], dtype=np.float32)
    u = np.asarray(inputs["u"], dtype=np.float32)
    v = np.asarray(inputs["v"], dtype=np.float32)
    s = np.asarray(inputs["s"], dtype=np.float32)
    h_in = np.asarray(inputs["h_in"], dtype=np.float32)
    h_out = np.asarray(inputs["h_out"], dtype=np.float32)

    bu = np.where(u >= 0, np.float32(1.0), np.float32(-1.0))
    bv = np.where(v >= 0, np.float32(1.0), np.float32(-1.0))
    W = (bv * s[None, :]) @ bu.T                 # [IN, OUT]
    W *= h_in[:, None]
    W *= h_out[None, :]
    W *= np.float32(0.25)                        # fp16 range headroom
    # w[j, p, k, c, o] = W[k*128+p, (j*WCHUNK+c)*128+o]
    w_t = np.ascontiguousarray(
        W.reshape(KI, P, J, WCHUNK, P).transpose(2, 1, 0, 3, 4)
    ).astype(np.float16)

    in_maps = []
    for i in range(N_CORES):
        xT_i = np.ascontiguousarray(x[i * TOK : (i + 1) * TOK, :].T).astype(
            np.float16
        )
        in_maps.append({"xT": xT_i, "w": w_t})

    _cache["in_maps"] = in_maps
    res = run_bass_kernel_spmd(
        nc, in_maps, core_ids=list(range(N_CORES)), trace=trace
    )

    y = np.empty((TOKENS, OUT), dtype=np.float32)
    for i in range(N_CORES):
        y[i * TOK : (i + 1) * TOK, :] = res.results[i]["yT"].T.astype(np.float32)
    y *= np.float32(4.0)
    return y, res


def kernel(**inputs):
    y, _ = _run(inputs, trace=False)
    return y


# revision 6
# speedup vs baseline: 2.1151x; 2.0959x over previous
"""Trainium2 Bass kernel for nn_LittleBitParallelLinear.

Computes y = ((x * h_in) @ sign(V)) * s @ sign(U).T * h_out with
sign(z) = +1 for z >= 0, -1 otherwise.

Strategy: fold the whole weight chain into a single dense matrix on the
host:  W = diag(h_in) @ sign(V) @ diag(s) @ sign(U).T @ diag(h_out).
Because RANK == IN/2 == OUT/2, the folded matmul x @ W has exactly the
same FLOP count as the two-matmul form (IN*OUT == IN*RANK + RANK*OUT),
but the device kernel becomes a single streaming GEMM with no Sign
activations, no intermediate stage, and half the dependency depth.

Token-parallel across 8 NeuronCores: core i handles tokens
[i*1024, (i+1)*1024); W is replicated. Inside each core the compute is
transposed (tokens on the matmul free dim):

    yT = W.T @ xT    [OUT, TOK]  fp16, streamed to DRAM

x and W are fp16 (W pre-scaled by 1/4 for range headroom; the host
multiplies the output by 4). PSUM accumulates fp32; the fp32->fp16
output cast rides the scalar engine. DMA instruction count is kept low
(x in 4 chunks, W/y in 2-block chunks) — per-DMA fixed costs on the SP
queue are significant on this hardware — and all DMAs stay on the SP
queue (the Activation-engine DGE queue measures ~2x slower).
"""

import numpy as np

P = 128
TOKENS, IN, OUT = 8192, 4096, 4096
N_CORES = 8
TOK = TOKENS // N_CORES   # 1024 tokens per core
KI = IN // P              # 32 contraction subtiles
MO = OUT // P             # 32 output row-blocks
FREE = 512                # PSUM bank free-dim (fp32)
NT = TOK // FREE          # 2 free chunks
XCHUNK = 8                # k-subtiles per x DMA
WCHUNK = 2                # output blocks per W DMA (and per y store)
J = MO // WCHUNK

_cache = {}


def _build():
    import concourse.bacc as bacc
    import concourse.mybir as mybir
    import concourse.tile as tile

    f32 = mybir.dt.float32
    f16 = mybir.dt.float16
    Copy = mybir.ActivationFunctionType.Copy

    nc = bacc.Bacc("TRN2", target_bir_lowering=False, debug=False)

    xT = nc.dram_tensor("xT", [IN, TOK], f16, kind="ExternalInput").ap()
    # W pre-tiled on host: w[j, p, k, c, o] = W[k*128+p, (j*WCHUNK+c)*128+o]
    w_ = nc.dram_tensor(
        "w", [J, P, KI, WCHUNK, P], f16, kind="ExternalInput"
    ).ap()
    yT = nc.dram_tensor("yT", [OUT, TOK], f16, kind="ExternalOutput").ap()

    with tile.TileContext(nc) as tc:
        with (
            tc.tile_pool(name="x", bufs=1) as xpool,
            tc.tile_pool(name="w", bufs=4) as wpool,
            tc.tile_pool(name="y", bufs=4) as ypool,
            tc.tile_pool(name="ps", bufs=8, space="PSUM") as psum,
        ):
            xs = xpool.tile([P, KI, TOK], f16)
            x3 = xT.rearrange("(k p) t -> p k t", p=P)
            y3 = yT.rearrange("(m p) t -> p m t", p=P)

            w_tiles = {}

            def load_w(j):
                wt = wpool.tile(
                    [P, KI, WCHUNK, P], f16, name=f"wt{j}", tag="wt"
                )
                nc.sync.dma_start(wt, w_[j])
                w_tiles[j] = wt

            load_w(0)
            next_wj = 1
            for kc in range(0, KI, XCHUNK):
                nc.sync.dma_start(
                    xs[:, kc : kc + XCHUNK], x3[:, kc : kc + XCHUNK]
                )
                if kc == 0 and next_wj < J:
                    load_w(next_wj)
                    next_wj += 1

            # Pair-major: k-major across the WCHUNK output blocks of each
            # W chunk (4 PSUM banks per pair, 2 pairs rotating through 8
            # banks) so the PE has 2x the runnable matmuls per x chunk
            # while x is still streaming in.
            for j in range(J):
                if next_wj <= min(j + 2, J - 1):
                    load_w(next_wj)
                    next_wj += 1
                wt = w_tiles[j]
                pss = {
                    (c, n): psum.tile(
                        [P, FREE], f32, name=f"ps_{j}_{c}_{n}", tag="ps"
                    )
                    for c in range(WCHUNK)
                    for n in range(NT)
                }
                for k in range(KI):
                    for c in range(WCHUNK):
                        for n in range(NT):
                            nc.tensor.matmul(
                                pss[(c, n)],
                                lhsT=wt[:, k, c],
                                rhs=xs[:, k, n * FREE : (n + 1) * FREE],
                                start=(k == 0),
                                stop=(k == KI - 1),
                            )
                yt = ypool.tile(
                    [P, WCHUNK, TOK], f16, name=f"yt_{j}", tag="yt"
                )
                last = j == J - 1
                for c in range(WCHUNK):
                    for n in range(NT):
                        nc.scalar.activation(
                            yt[:, c, n * FREE : (n + 1) * FREE],
                            pss[(c, n)],
                            Copy,
                        )
                    if last:
                        nc.sync.dma_start(
                            y3[:, WCHUNK * j + c : WCHUNK * j + c + 1],
                            yt[:, c : c + 1],
                        )
                if not last:
                    nc.sync.dma_start(
                        y3[:, WCHUNK * j : WCHUNK * (j + 1)], yt
                    )
                w_tiles.pop(j)

    nc.compile()
    return nc


def _run(inputs, trace=False):
    from concourse.bass_utils import run_bass_kernel_spmd

    if "nc" not in _cache:
        _cache["nc"] = _build()
    nc = _cache["nc"]

    x = np.asarray(inputs["x"], dtype=np.float32)
    u = np.asarray(inputs["u"], dtype=np.float32)
    v = np.asarray(inputs["v"], dtype=np.float32)
    s = np.asarray(inputs["s"], dtype=np.float32)
    h_in = np.asarray(inputs["h_in"], dtype=np.float32)
    h_out = np.asarray(inputs["h_out"], dtype=np.float32)

    bu = np.where(u >= 0, np.float32(1.0), np.float32(-1.0))
    bv = np.where(v >= 0, np.float32(1.0), np.float32(-1.0))
    W = (bv * s[None, :]) @ bu.T                 # [IN, OUT]
    W *= h_in[:, None]
    W *= h_out[None, :]
    W *= np.float32(0.25)                        # fp16 range headroom
    # w[j, p, k, c, o] = W[k*128+p, (j*WCHUNK+c)*128+o]
    w_t = np.ascontiguousarray(
        W.reshape(KI, P, J, WCHUNK, P).transpose(2, 1, 0, 3, 4)
    ).astype(np.float16)

    in_maps = []
    for i in range(N_CORES):
        xT_i = np.ascontiguousarray(x[i * TOK : (i + 1) * TOK, :].T).astype(
            np.float16
        )
        in_maps.append({"xT": xT_i, "w": w_t})

    _cache["in_maps"] = in_maps
    res = run_bass_kernel_spmd(
        nc, in_maps, core_ids=list(range(N_CORES)), trace=trace
    )

    y = np.empty((TOKENS, OUT), dtype=np.float32)
    for i in range(N_CORES):
        y[i * TOK : (i + 1) * TOK, :] = res.results[i]["yT"].T.astype(np.float32)
    y *= np.float32(4.0)
    return y, res


def kernel(**inputs):
    y, _ = _run(inputs, trace=False)
    return y
